# revision 17
# baseline (speedup 1.0000x reference)
"""Trainium2 Bass kernel for im2col conv2d + bias + channel-pack.

Semantics (matches the reference):
    out[c, w] = sum_k enc_x[w, k] * weight[c, k] + bias[c],  flattened to [C*W].

Strategy:
  - Shard the window dimension W=1048576 across 8 cores (131072 windows each).
  - Host-side: transpose enc_x to [K, W] (so the contraction dim K=49 lands on
    SBUF partitions) and cast to fp16 (halves HBM traffic; PE accumulates fp32).
  - Device-side: stationary operand is a block-diagonal [2K, 2C] weight matrix,
    so each matmul computes TWO 512-window chunks at once and the output tile
    occupies 64 partitions (keeps the scalar-engine bias/copy off the critical
    path). Bias is fused into the PSUM->SBUF copy via activation(Identity, bias).
  - Memory-bound regime: per-core HBM traffic = 12.8 MB in + 16.8 MB out.
"""

import os

import numpy as np

K = 49
C = 32
WINDOWS_NB = 1048576
N_CORES = 8
W_CORE = WINDOWS_NB // N_CORES  # 131072

# Device tiling parameters (full-size problem).
F = 16384  # windows per half input tile  (x_tile is [2K, F], covers 2F windows)
GROUP = 2048  # psum tile free dim (4 MM pairs of 512)
NMM = 512  # matmul moving free dim (one PSUM bank of fp32)

_PROGRAM_CACHE: dict = {}
LAST_RESULT = None  # BassKernelResults of the most recent run (for test harness)


def build_program(w_core=W_CORE, f=F, group=GROUP, nmm=NMM):
    import concourse.tile as tile
    from concourse import bacc, mybir

    pair = 2 * nmm  # windows-per-half covered by one concurrent MM pair
    assert w_core % (2 * f) == 0 and f % (4 * pair) == 0 and group == 4 * nmm
    n_outer = w_core // (2 * f)
    npair = f // pair  # MM pairs per outer iteration

    nc = bacc.Bacc("TRN2", debug=False, num_devices=N_CORES)
    xt = nc.dram_tensor("xt", [K, w_core], mybir.dt.float16, kind="ExternalInput")
    # Block-diag weights duplicated into both 64-column halves of the PE
    # array: cols [64j..64j+31] = W for k-rows 0..48, cols [64j+32..64j+63]
    # = W for k-rows 49..97. Two matmuls on different column groups run
    # concurrently and fill all 128 PSUM partitions.
    w4 = nc.dram_tensor("w4", [2 * K, 4 * C], mybir.dt.float16, kind="ExternalInput")
    br = nc.dram_tensor("br", [4 * C, 1], mybir.dt.float32, kind="ExternalInput")
    # fp16 output (upcast on host): halves HBM write traffic, which is the
    # dominant cost in this memory-bound kernel.
    out = nc.dram_tensor("out", [C, w_core], mybir.dt.float16, kind="ExternalOutput")

    with tile.TileContext(nc) as tc:
        with tc.tile_pool(name="const", bufs=1) as cpool, \
             tc.tile_pool(name="xin", bufs=3) as xpool, \
             tc.tile_pool(name="osb", bufs=3) as opool, \
             tc.tile_pool(name="ps", bufs=2, space="PSUM") as ppool:
            w_sb = cpool.tile([2 * K, 4 * C], mybir.dt.float16)
            nc.sync.dma_start(out=w_sb, in_=w4.ap())
            b_sb = cpool.tile([4 * C, 1], mybir.dt.float32)
            nc.sync.dma_start(out=b_sb, in_=br.ap())

            xt_ap = xt.ap()
            # out element [c, w]; w = i*2f + j*f + p*2*nmm + h*nmm + t
            out_r = out.ap().rearrange(
                "c (i j p h t) -> i j h c p t", i=n_outer, j=2, p=npair, h=2, t=nmm
            )

            for it in range(n_outer):
                a0 = it * 2 * f
                x_tile = xpool.tile([2 * K, f], mybir.dt.float16)
                # Split each half-load into 48+1 rows: HWDGE fans a DMA's
                # descriptors over the largest engine count dividing the outer
                # dim (49 -> only 7 engines; 48 -> all 16).
                nc.sync.dma_start(out=x_tile[0:48, :], in_=xt_ap[0:48, a0:a0 + f])
                nc.sync.dma_start(out=x_tile[48:K, :], in_=xt_ap[48:K, a0:a0 + f])
                nc.sync.dma_start(out=x_tile[K:K + 48, :], in_=xt_ap[0:48, a0 + f:a0 + 2 * f])
                nc.sync.dma_start(out=x_tile[K + 48:2 * K, :], in_=xt_ap[48:K, a0 + f:a0 + 2 * f])
                o_tile = opool.tile([4 * C, f // 2], mybir.dt.float16)
                for q in range(npair // 4):
                    ps = ppool.tile([4 * C, group], mybir.dt.float32)
                    for r in range(4):
                        p = 4 * q + r
                        # concurrent MM pair on PE column groups 0-1 / 2-3
                        nc.tensor.matmul(
                            ps[0:2 * C, r * nmm:(r + 1) * nmm],
                            w_sb[:, 0:2 * C],
                            x_tile[:, p * pair:p * pair + nmm],
                            start=True,
                            stop=True,
                            tile_position=(0, 0),
                        )
                        nc.tensor.matmul(
                            ps[2 * C:4 * C, r * nmm:(r + 1) * nmm],
                            w_sb[:, 2 * C:4 * C],
                            x_tile[:, p * pair + nmm:(p + 1) * pair],
                            start=True,
                            stop=True,
                            tile_position=(0, 2 * C),
                        )
                    nc.scalar.activation(
                        o_tile[:, q * group:(q + 1) * group],
                        ps,
                        mybir.ActivationFunctionType.Identity,
                        bias=b_sb,
                        scale=1.0,
                    )
                # One DMA per 32-partition block: DRAM-side outer dim 32 (c)
                # spreads descriptors over all 16 engines. Stores ride the
                # scalar-engine HWDGE ring so they can't head-of-line-block
                # the input loads on the sync ring.
                for jj in range(4):
                    nc.scalar.dma_start(
                        out=out_r[it, jj & 1, jj >> 1],
                        in_=o_tile[jj * C:(jj + 1) * C, :].rearrange(
                            "c (p t) -> c p t", t=nmm
                        ),
                    )
    nc.compile()
    return nc


def _get_program():
    key = (W_CORE, F, GROUP, NMM)
    if key not in _PROGRAM_CACHE:
        _PROGRAM_CACHE[key] = build_program()
    return _PROGRAM_CACHE[key]


def prepare_inputs(enc_x, weight, bias):
    """Host-side prep: per-core transposed fp16 shards + block-diag weights."""
    enc_x = np.asarray(enc_x, dtype=np.float32)
    weight = np.asarray(weight, dtype=np.float32)
    bias = np.asarray(bias, dtype=np.float32)

    wflat = weight.reshape(C, K)
    wt16 = wflat.T.astype(np.float16)
    w4 = np.zeros((2 * K, 4 * C), dtype=np.float16)
    for j in range(2):
        w4[0:K, 2 * j * C:(2 * j + 1) * C] = wt16
        w4[K:2 * K, (2 * j + 1) * C:(2 * j + 2) * C] = wt16
    br = np.tile(bias, 4)[:, None].astype(np.float32)

    x16 = enc_x.astype(np.float16)
    shards = [
        np.ascontiguousarray(x16[i * W_CORE:(i + 1) * W_CORE].T)
        for i in range(N_CORES)
    ]
    return shards, w4, br


def kernel(enc_x, weight, bias, windows_nb=None):
    global LAST_RESULT
    from concourse import bass_utils

    shards, w4, br = prepare_inputs(enc_x, weight, bias)
    nc = _get_program()
    in_maps = [{"xt": shards[i], "w4": w4, "br": br} for i in range(N_CORES)]
    trace = bool(int(os.environ.get("BASS_KERNEL_TRACE", "0")))
    tmpdir = os.environ.get("BASS_KERNEL_TMPDIR") or None
    res = bass_utils.run_bass_kernel_spmd(
        nc, in_maps, core_ids=list(range(N_CORES)), trace=trace, tmpdir=tmpdir
    )
    LAST_RESULT = res
    outs = [res.results[i]["out"] for i in range(N_CORES)]
    return np.concatenate(outs, axis=1).astype(np.float32).reshape(-1)


# revision 20
# speedup vs baseline: 1.1658x; 1.1658x over previous
"""Trainium2 Bass kernel for im2col conv2d + bias + channel-pack.

Semantics (matches the reference):
    out[c, w] = sum_k enc_x[w, k] * weight[c, k] + bias[c],  flattened to [C*W].

Strategy:
  - Shard the window dimension W=1048576 across 8 cores (131072 windows each).
  - Host-side: transpose enc_x to [K, W] (so the contraction dim K=49 lands on
    SBUF partitions) and cast to fp16 (halves HBM traffic; PE accumulates fp32).
  - Device-side: stationary operand is a block-diagonal [2K, 2C] weight matrix,
    so each matmul computes TWO 512-window chunks at once and the output tile
    occupies 64 partitions (keeps the scalar-engine bias/copy off the critical
    path). Bias is fused into the PSUM->SBUF copy via activation(Identity, bias).
  - Memory-bound regime: per-core HBM traffic = 12.8 MB in + 16.8 MB out.
"""

import os

import numpy as np

K = 49
C = 32
WINDOWS_NB = 1048576
N_CORES = 8
W_CORE = WINDOWS_NB // N_CORES  # 131072

# Device tiling parameters (full-size problem).
F = 8192  # windows per half input tile  (x_tile is [2K, F], covers 2F windows)
GROUP = 2048  # psum tile free dim (4 MM pairs of 512)
NMM = 512  # matmul moving free dim (one PSUM bank of fp32)

_PROGRAM_CACHE: dict = {}
LAST_RESULT = None  # BassKernelResults of the most recent run (for test harness)


def build_program(w_core=W_CORE, f=F, group=GROUP, nmm=NMM):
    import concourse.tile as tile
    from concourse import bacc, mybir

    pair = 2 * nmm  # windows-per-half covered by one concurrent MM pair
    assert w_core % (2 * f) == 0 and f % (4 * pair) == 0 and group == 4 * nmm
    n_outer = w_core // (2 * f)
    npair = f // pair  # MM pairs per outer iteration

    nc = bacc.Bacc("TRN2", debug=False, num_devices=N_CORES)
    xt = nc.dram_tensor("xt", [K, w_core], mybir.dt.float16, kind="ExternalInput")
    # Block-diag weights duplicated into both 64-column halves of the PE
    # array: cols [64j..64j+31] = W for k-rows 0..48, cols [64j+32..64j+63]
    # = W for k-rows 49..97. Two matmuls on different column groups run
    # concurrently and fill all 128 PSUM partitions.
    w4 = nc.dram_tensor("w4", [2 * K, 4 * C], mybir.dt.float16, kind="ExternalInput")
    br = nc.dram_tensor("br", [4 * C, 1], mybir.dt.float32, kind="ExternalInput")
    # fp16 output (upcast on host): halves HBM write traffic, which is the
    # dominant cost in this memory-bound kernel.
    out = nc.dram_tensor("out", [C, w_core], mybir.dt.float16, kind="ExternalOutput")

    with tile.TileContext(nc) as tc:
        with tc.tile_pool(name="const", bufs=1) as cpool, \
             tc.tile_pool(name="xin", bufs=3) as xpool, \
             tc.tile_pool(name="osb", bufs=3) as opool, \
             tc.tile_pool(name="ps", bufs=2, space="PSUM") as ppool:
            w_sb = cpool.tile([2 * K, 4 * C], mybir.dt.float16)
            nc.sync.dma_start(out=w_sb, in_=w4.ap())
            b_sb = cpool.tile([4 * C, 1], mybir.dt.float32)
            nc.sync.dma_start(out=b_sb, in_=br.ap())

            xt_ap = xt.ap()
            # out element [c, w]; w = i*2f + j*f + p*2*nmm + h*nmm + t
            out_r = out.ap().rearrange(
                "c (i j p h t) -> i j h c p t", i=n_outer, j=2, p=npair, h=2, t=nmm
            )

            for it in range(n_outer):
                a0 = it * 2 * f
                x_tile = xpool.tile([2 * K, f], mybir.dt.float16)
                # Split each half-load into 48+1 rows: HWDGE fans a DMA's
                # descriptors over the largest engine count dividing the outer
                # dim (49 -> only 7 engines; 48 -> all 16). The big loads ride
                # the scalar HWDGE ring (lower queue priority: stores preempt
                # them instead of starving); the 1-row remainders go through
                # SWDGE so they never stall a HWDGE ring.
                nc.scalar.dma_start(out=x_tile[0:48, :], in_=xt_ap[0:48, a0:a0 + f])
                nc.gpsimd.dma_start(out=x_tile[48:K, :], in_=xt_ap[48:K, a0:a0 + f])
                nc.scalar.dma_start(out=x_tile[K:K + 48, :], in_=xt_ap[0:48, a0 + f:a0 + 2 * f])
                nc.gpsimd.dma_start(out=x_tile[K + 48:2 * K, :], in_=xt_ap[48:K, a0 + f:a0 + 2 * f])
                o_tile = opool.tile([4 * C, f // 2], mybir.dt.float16)
                for q in range(npair // 4):
                    ps = ppool.tile([4 * C, group], mybir.dt.float32)
                    for r in range(4):
                        p = 4 * q + r
                        # concurrent MM pair on PE column groups 0-1 / 2-3
                        nc.tensor.matmul(
                            ps[0:2 * C, r * nmm:(r + 1) * nmm],
                            w_sb[:, 0:2 * C],
                            x_tile[:, p * pair:p * pair + nmm],
                            start=True,
                            stop=True,
                            tile_position=(0, 0),
                        )
                        nc.tensor.matmul(
                            ps[2 * C:4 * C, r * nmm:(r + 1) * nmm],
                            w_sb[:, 2 * C:4 * C],
                            x_tile[:, p * pair + nmm:(p + 1) * pair],
                            start=True,
                            stop=True,
                            tile_position=(0, 2 * C),
                        )
                    nc.scalar.activation(
                        o_tile[:, q * group:(q + 1) * group],
                        ps,
                        mybir.ActivationFunctionType.Identity,
                        bias=b_sb,
                        scale=1.0,
                    )
                # One DMA per 32-partition block: DRAM-side outer dim 32 (c)
                # spreads descriptors over all 16 engines. Stores ride the
                # sync HWDGE ring (higher queue priority than the scalar
                # ring): they are throttled by compute anyway, so they
                # preempt the input stream briefly instead of being starved
                # by it.
                for jj in range(4):
                    nc.sync.dma_start(
                        out=out_r[it, jj & 1, jj >> 1],
                        in_=o_tile[jj * C:(jj + 1) * C, :].rearrange(
                            "c (p t) -> c p t", t=nmm
                        ),
                    )
    nc.compile()
    return nc


def _get_program():
    key = (W_CORE, F, GROUP, NMM)
    if key not in _PROGRAM_CACHE:
        _PROGRAM_CACHE[key] = build_program()
    return _PROGRAM_CACHE[key]


def prepare_inputs(enc_x, weight, bias):
    """Host-side prep: per-core transposed fp16 shards + block-diag weights."""
    enc_x = np.asarray(enc_x, dtype=np.float32)
    weight = np.asarray(weight, dtype=np.float32)
    bias = np.asarray(bias, dtype=np.float32)

    wflat = weight.reshape(C, K)
    wt16 = wflat.T.astype(np.float16)
    w4 = np.zeros((2 * K, 4 * C), dtype=np.float16)
    for j in range(2):
        w4[0:K, 2 * j * C:(2 * j + 1) * C] = wt16
        w4[K:2 * K, (2 * j + 1) * C:(2 * j + 2) * C] = wt16
    br = np.tile(bias, 4)[:, None].astype(np.float32)

    x16 = enc_x.astype(np.float16)
    shards = [
        np.ascontiguousarray(x16[i * W_CORE:(i + 1) * W_CORE].T)
        for i in range(N_CORES)
    ]
    return shards, w4, br


def kernel(enc_x, weight, bias, windows_nb=None):
    global LAST_RESULT
    from concourse import bass_utils

    shards, w4, br = prepare_inputs(enc_x, weight, bias)
    nc = _get_program()
    in_maps = [{"xt": shards[i], "w4": w4, "br": br} for i in range(N_CORES)]
    trace = bool(int(os.environ.get("BASS_KERNEL_TRACE", "0")))
    tmpdir = os.environ.get("BASS_KERNEL_TMPDIR") or None
    res = bass_utils.run_bass_kernel_spmd(
        nc, in_maps, core_ids=list(range(N_CORES)), trace=trace, tmpdir=tmpdir
    )
    LAST_RESULT = res
    outs = [res.results[i]["out"] for i in range(N_CORES)]
    return np.concatenate(outs, axis=1).astype(np.float32).reshape(-1)


# revision 21
# speedup vs baseline: 1.2130x; 1.0404x over previous
"""Trainium2 Bass kernel for im2col conv2d + bias + channel-pack.

Semantics (matches the reference):
    out[c, w] = sum_k enc_x[w, k] * weight[c, k] + bias[c],  flattened to [C*W].

Strategy:
  - Shard the window dimension W=1048576 across 8 cores (131072 windows each).
  - Host-side: transpose enc_x to [K, W] (so the contraction dim K=49 lands on
    SBUF partitions) and cast to fp16 (halves HBM traffic; PE accumulates fp32).
  - Device-side: stationary operand is a block-diagonal [2K, 2C] weight matrix,
    so each matmul computes TWO 512-window chunks at once and the output tile
    occupies 64 partitions (keeps the scalar-engine bias/copy off the critical
    path). Bias is fused into the PSUM->SBUF copy via activation(Identity, bias).
  - Memory-bound regime: per-core HBM traffic = 12.8 MB in + 16.8 MB out.
"""

import os

import numpy as np

K = 49
C = 32
WINDOWS_NB = 1048576
N_CORES = 8
W_CORE = WINDOWS_NB // N_CORES  # 131072

# Device tiling parameters (full-size problem).
F = 8192  # windows per half input tile  (x_tile is [2K, F], covers 2F windows)
GROUP = 2048  # psum tile free dim (4 MM pairs of 512)
NMM = 512  # matmul moving free dim (one PSUM bank of fp32)

_PROGRAM_CACHE: dict = {}
LAST_RESULT = None  # BassKernelResults of the most recent run (for test harness)


def build_program(w_core=W_CORE, f=F, group=GROUP, nmm=NMM):
    import concourse.tile as tile
    from concourse import bacc, mybir

    pair = 2 * nmm  # windows-per-half covered by one concurrent MM pair
    assert w_core % (2 * f) == 0 and f % (4 * pair) == 0 and group == 4 * nmm
    n_outer = w_core // (2 * f)
    npair = f // pair  # MM pairs per outer iteration

    nc = bacc.Bacc("TRN2", debug=False, num_devices=N_CORES)
    xt = nc.dram_tensor("xt", [K, w_core], mybir.dt.float16, kind="ExternalInput")
    # Block-diag weights duplicated into both 64-column halves of the PE
    # array: cols [64j..64j+31] = W for k-rows 0..48, cols [64j+32..64j+63]
    # = W for k-rows 49..97. Two matmuls on different column groups run
    # concurrently and fill all 128 PSUM partitions.
    w4 = nc.dram_tensor("w4", [2 * K, 4 * C], mybir.dt.float16, kind="ExternalInput")
    br = nc.dram_tensor("br", [4 * C, 1], mybir.dt.float32, kind="ExternalInput")
    # fp16 output (upcast on host): halves HBM write traffic, which is the
    # dominant cost in this memory-bound kernel.
    out = nc.dram_tensor("out", [C, w_core], mybir.dt.float16, kind="ExternalOutput")

    with tile.TileContext(nc) as tc:
        with tc.tile_pool(name="const", bufs=1) as cpool, \
             tc.tile_pool(name="xin", bufs=3) as xpool, \
             tc.tile_pool(name="osb", bufs=3) as opool, \
             tc.tile_pool(name="ps", bufs=2, space="PSUM") as ppool:
            w_sb = cpool.tile([2 * K, 4 * C], mybir.dt.float16)
            nc.sync.dma_start(out=w_sb, in_=w4.ap())
            b_sb = cpool.tile([4 * C, 1], mybir.dt.float32)
            nc.sync.dma_start(out=b_sb, in_=br.ap())

            xt_ap = xt.ap()
            # out element [c, w]; w = i*2f + j*f + p*2*nmm + h*nmm + t
            out_r = out.ap().rearrange(
                "c (i j p h t) -> i j h c p t", i=n_outer, j=2, p=npair, h=2, t=nmm
            )

            for it in range(n_outer):
                a0 = it * 2 * f
                x_tile = xpool.tile([2 * K, f], mybir.dt.float16)
                # Input rides two independent descriptor generators in
                # parallel: half0 on the scalar HWDGE ring (48+1 row split so
                # descriptors fan over all 16 engines: HWDGE uses the largest
                # engine count dividing the outer dim, and 49 -> only 7),
                # half1 on the gpsimd SWDGE path (partition-port spray, no
                # split needed). Doubles input instruction pacing.
                nc.scalar.dma_start(out=x_tile[0:48, :], in_=xt_ap[0:48, a0:a0 + f])
                nc.gpsimd.dma_start(out=x_tile[48:K, :], in_=xt_ap[48:K, a0:a0 + f])
                nc.gpsimd.dma_start(out=x_tile[K:2 * K, :], in_=xt_ap[:, a0 + f:a0 + 2 * f])
                o_tile = opool.tile([4 * C, f // 2], mybir.dt.float16)
                for q in range(npair // 4):
                    ps = ppool.tile([4 * C, group], mybir.dt.float32)
                    for r in range(4):
                        p = 4 * q + r
                        # concurrent MM pair on PE column groups 0-1 / 2-3
                        nc.tensor.matmul(
                            ps[0:2 * C, r * nmm:(r + 1) * nmm],
                            w_sb[:, 0:2 * C],
                            x_tile[:, p * pair:p * pair + nmm],
                            start=True,
                            stop=True,
                            tile_position=(0, 0),
                        )
                        nc.tensor.matmul(
                            ps[2 * C:4 * C, r * nmm:(r + 1) * nmm],
                            w_sb[:, 2 * C:4 * C],
                            x_tile[:, p * pair + nmm:(p + 1) * pair],
                            start=True,
                            stop=True,
                            tile_position=(0, 2 * C),
                        )
                    nc.scalar.activation(
                        o_tile[:, q * group:(q + 1) * group],
                        ps,
                        mybir.ActivationFunctionType.Identity,
                        bias=b_sb,
                        scale=1.0,
                    )
                # One DMA per 32-partition block: DRAM-side outer dim 32 (c)
                # spreads descriptors over all 16 engines. Stores ride the
                # sync HWDGE ring (higher queue priority than the scalar
                # ring): they are throttled by compute anyway, so they
                # preempt the input stream briefly instead of being starved
                # by it.
                for jj in range(4):
                    nc.sync.dma_start(
                        out=out_r[it, jj & 1, jj >> 1],
                        in_=o_tile[jj * C:(jj + 1) * C, :].rearrange(
                            "c (p t) -> c p t", t=nmm
                        ),
                    )
    nc.compile()
    return nc


def _get_program():
    key = (W_CORE, F, GROUP, NMM)
    if key not in _PROGRAM_CACHE:
        _PROGRAM_CACHE[key] = build_program()
    return _PROGRAM_CACHE[key]


def prepare_inputs(enc_x, weight, bias):
    """Host-side prep: per-core transposed fp16 shards + block-diag weights."""
    enc_x = np.asarray(enc_x, dtype=np.float32)
    weight = np.asarray(weight, dtype=np.float32)
    bias = np.asarray(bias, dtype=np.float32)

    wflat = weight.reshape(C, K)
    wt16 = wflat.T.astype(np.float16)
    w4 = np.zeros((2 * K, 4 * C), dtype=np.float16)
    for j in range(2):
        w4[0:K, 2 * j * C:(2 * j + 1) * C] = wt16
        w4[K:2 * K, (2 * j + 1) * C:(2 * j + 2) * C] = wt16
    br = np.tile(bias, 4)[:, None].astype(np.float32)

    x16 = enc_x.astype(np.float16)
    shards = [
        np.ascontiguousarray(x16[i * W_CORE:(i + 1) * W_CORE].T)
        for i in range(N_CORES)
    ]
    return shards, w4, br


def kernel(enc_x, weight, bias, windows_nb=None):
    global LAST_RESULT
    from concourse import bass_utils

    shards, w4, br = prepare_inputs(enc_x, weight, bias)
    nc = _get_program()
    in_maps = [{"xt": shards[i], "w4": w4, "br": br} for i in range(N_CORES)]
    trace = bool(int(os.environ.get("BASS_KERNEL_TRACE", "0")))
    tmpdir = os.environ.get("BASS_KERNEL_TMPDIR") or None
    res = bass_utils.run_bass_kernel_spmd(
        nc, in_maps, core_ids=list(range(N_CORES)), trace=trace, tmpdir=tmpdir
    )
    LAST_RESULT = res
    outs = [res.results[i]["out"] for i in range(N_CORES)]
    return np.concatenate(outs, axis=1).astype(np.float32).reshape(-1)


# revision 22
# speedup vs baseline: 1.2618x; 1.0402x over previous
"""Trainium2 Bass kernel for im2col conv2d + bias + channel-pack.

Semantics (matches the reference):
    out[c, w] = sum_k enc_x[w, k] * weight[c, k] + bias[c],  flattened to [C*W].

Strategy:
  - Shard the window dimension W=1048576 across 8 cores (131072 windows each).
  - Host-side: transpose enc_x to [K, W] (so the contraction dim K=49 lands on
    SBUF partitions) and cast to fp16 (halves HBM traffic; PE accumulates fp32).
  - Device-side: stationary operand is a block-diagonal [2K, 2C] weight matrix,
    so each matmul computes TWO 512-window chunks at once and the output tile
    occupies 64 partitions (keeps the scalar-engine bias/copy off the critical
    path). Bias is fused into the PSUM->SBUF copy via activation(Identity, bias).
  - Memory-bound regime: per-core HBM traffic = 12.8 MB in + 16.8 MB out.
"""

import os

import numpy as np

K = 49
C = 32
WINDOWS_NB = 1048576
N_CORES = 8
W_CORE = WINDOWS_NB // N_CORES  # 131072

# Device tiling parameters (full-size problem).
F = 8192  # windows per half input tile  (x_tile is [2K, F], covers 2F windows)
GROUP = 2048  # psum tile free dim (4 MM pairs of 512)
NMM = 512  # matmul moving free dim (one PSUM bank of fp32)

_PROGRAM_CACHE: dict = {}
LAST_RESULT = None  # BassKernelResults of the most recent run (for test harness)


def build_program(w_core=W_CORE, f=F, group=GROUP, nmm=NMM):
    import concourse.tile as tile
    from concourse import bacc, mybir

    pair = 2 * nmm  # windows-per-half covered by one concurrent MM pair
    assert w_core % (2 * f) == 0 and f % (4 * pair) == 0 and group == 4 * nmm
    n_outer = w_core // (2 * f)
    npair = f // pair  # MM pairs per outer iteration

    nc = bacc.Bacc("TRN2", debug=False, num_devices=N_CORES)
    xt = nc.dram_tensor("xt", [K, w_core], mybir.dt.float16, kind="ExternalInput")
    # Block-diag weights duplicated into both 64-column halves of the PE
    # array: cols [64j..64j+31] = W for k-rows 0..48, cols [64j+32..64j+63]
    # = W for k-rows 49..97. Two matmuls on different column groups run
    # concurrently and fill all 128 PSUM partitions.
    w4 = nc.dram_tensor("w4", [2 * K, 4 * C], mybir.dt.float16, kind="ExternalInput")
    br = nc.dram_tensor("br", [4 * C, 1], mybir.dt.float32, kind="ExternalInput")
    # fp16 output (upcast on host): halves HBM write traffic, which is the
    # dominant cost in this memory-bound kernel.
    out = nc.dram_tensor("out", [C, w_core], mybir.dt.float16, kind="ExternalOutput")

    with tile.TileContext(nc) as tc:
        with tc.tile_pool(name="const", bufs=1) as cpool, \
             tc.tile_pool(name="xin", bufs=3) as xpool, \
             tc.tile_pool(name="osb", bufs=3) as opool, \
             tc.tile_pool(name="ps", bufs=2, space="PSUM") as ppool:
            w_sb = cpool.tile([2 * K, 4 * C], mybir.dt.float16)
            nc.sync.dma_start(out=w_sb, in_=w4.ap())
            b_sb = cpool.tile([4 * C, 1], mybir.dt.float32)
            nc.sync.dma_start(out=b_sb, in_=br.ap())

            xt_ap = xt.ap()
            # out element [c, w]; w = i*2f + j*f + p*2*nmm + h*nmm + t
            out_r = out.ap().rearrange(
                "c (i j p h t) -> i j h c p t", i=n_outer, j=2, p=npair, h=2, t=nmm
            )

            for it in range(n_outer):
                a0 = it * 2 * f
                x_tile = xpool.tile([2 * K, f], mybir.dt.float16)
                # Input rides two independent descriptor generators in
                # parallel: half0 on the scalar HWDGE ring (48+1 row split so
                # descriptors fan over all 16 engines: HWDGE uses the largest
                # engine count dividing the outer dim, and 49 -> only 7),
                # half1 on the gpsimd SWDGE path (partition-port spray, no
                # split needed). Doubles input instruction pacing.
                nc.scalar.dma_start(out=x_tile[0:48, :], in_=xt_ap[0:48, a0:a0 + f])
                nc.gpsimd.dma_start(out=x_tile[48:K, :], in_=xt_ap[48:K, a0:a0 + f])
                if it == 0:
                    # SWDGE is slow to warm up (Q7 startup); load the first
                    # tile's second half on the already-hot scalar HWDGE ring
                    # so the first matmul isn't delayed.
                    nc.scalar.dma_start(out=x_tile[K:K + 48, :], in_=xt_ap[0:48, a0 + f:a0 + 2 * f])
                    nc.gpsimd.dma_start(out=x_tile[K + 48:2 * K, :], in_=xt_ap[48:K, a0 + f:a0 + 2 * f])
                else:
                    nc.gpsimd.dma_start(out=x_tile[K:2 * K, :], in_=xt_ap[:, a0 + f:a0 + 2 * f])
                o_tile = opool.tile([4 * C, f // 2], mybir.dt.float16)
                for q in range(npair // 4):
                    ps = ppool.tile([4 * C, group], mybir.dt.float32)
                    for r in range(4):
                        p = 4 * q + r
                        # concurrent MM pair on PE column groups 0-1 / 2-3
                        nc.tensor.matmul(
                            ps[0:2 * C, r * nmm:(r + 1) * nmm],
                            w_sb[:, 0:2 * C],
                            x_tile[:, p * pair:p * pair + nmm],
                            start=True,
                            stop=True,
                            tile_position=(0, 0),
                        )
                        nc.tensor.matmul(
                            ps[2 * C:4 * C, r * nmm:(r + 1) * nmm],
                            w_sb[:, 2 * C:4 * C],
                            x_tile[:, p * pair + nmm:(p + 1) * pair],
                            start=True,
                            stop=True,
                            tile_position=(0, 2 * C),
                        )
                    nc.scalar.activation(
                        o_tile[:, q * group:(q + 1) * group],
                        ps,
                        mybir.ActivationFunctionType.Identity,
                        bias=b_sb,
                        scale=1.0,
                    )
                # One DMA per 32-partition block: DRAM-side outer dim 32 (c)
                # spreads descriptors over all 16 engines. Stores ride the
                # sync HWDGE ring (higher queue priority than the scalar
                # ring): they are throttled by compute anyway, so they
                # preempt the input stream briefly instead of being starved
                # by it.
                for jj in range(4):
                    nc.sync.dma_start(
                        out=out_r[it, jj & 1, jj >> 1],
                        in_=o_tile[jj * C:(jj + 1) * C, :].rearrange(
                            "c (p t) -> c p t", t=nmm
                        ),
                    )
    nc.compile()
    return nc


def _get_program():
    key = (W_CORE, F, GROUP, NMM)
    if key not in _PROGRAM_CACHE:
        _PROGRAM_CACHE[key] = build_program()
    return _PROGRAM_CACHE[key]


def prepare_inputs(enc_x, weight, bias):
    """Host-side prep: per-core transposed fp16 shards + block-diag weights."""
    enc_x = np.asarray(enc_x, dtype=np.float32)
    weight = np.asarray(weight, dtype=np.float32)
    bias = np.asarray(bias, dtype=np.float32)

    wflat = weight.reshape(C, K)
    wt16 = wflat.T.astype(np.float16)
    w4 = np.zeros((2 * K, 4 * C), dtype=np.float16)
    for j in range(2):
        w4[0:K, 2 * j * C:(2 * j + 1) * C] = wt16
        w4[K:2 * K, (2 * j + 1) * C:(2 * j + 2) * C] = wt16
    br = np.tile(bias, 4)[:, None].astype(np.float32)

    x16 = enc_x.astype(np.float16)
    shards = [
        np.ascontiguousarray(x16[i * W_CORE:(i + 1) * W_CORE].T)
        for i in range(N_CORES)
    ]
    return shards, w4, br


def kernel(enc_x, weight, bias, windows_nb=None):
    global LAST_RESULT
    from concourse import bass_utils

    shards, w4, br = prepare_inputs(enc_x, weight, bias)
    nc = _get_program()
    in_maps = [{"xt": shards[i], "w4": w4, "br": br} for i in range(N_CORES)]
    trace = bool(int(os.environ.get("BASS_KERNEL_TRACE", "0")))
    tmpdir = os.environ.get("BASS_KERNEL_TMPDIR") or None
    res = bass_utils.run_bass_kernel_spmd(
        nc, in_maps, core_ids=list(range(N_CORES)), trace=trace, tmpdir=tmpdir
    )
    LAST_RESULT = res
    outs = [res.results[i]["out"] for i in range(N_CORES)]
    return np.concatenate(outs, axis=1).astype(np.float32).reshape(-1)
